# revision 1
# baseline (speedup 1.0000x reference)
"""BiAffine layer kernel for 8 Trainium2 NeuronCores.

Reference computation (per batch b):
  s = relu(x @ sW.T + sb)                  [L, E]
  t = relu(x @ tW.T + tb)                  [L, E]
  key = (s @ blW.T).reshape(L, E, N)
  out1[i, n, l] = sum_e key[i, e, n] * t[l, e]
  su = s @ Wu.T ; tv = t @ Wv.T            (Wu, Wv = f2W[:, :E], f2W[:, E:])
  h[i, j, :] = relu(su[i] + tv[j] + f2b)
  out2[i, n, j] = sum_e h[i, j, e] * f3W[n, e] + f3b[n]
  out = out1 + out2                        [L, N, L]

Sharding: 8 cores = 2 batches x 4 blocks of 128 source positions (i).

Octet layout: one PSUM bank [128, 512] holds EIGHT i's: 4 col-groups at
32-aligned offsets, 2 i's packed per group (rows 32k + 12s + n, 8 pad
rows per group).  out1: M=32 matmuls from a zero-padded fp16 key tensor
(also initializes the bank); out2: M=24 matmuls with zero-block-padded
f3W stationaries, 4-way PE column-group concurrency throughout.

h production per octet position p = i%8:
  p=0..5 -> DVE: h' = max(tv, -su') one fp16 tensor_tensor per i with an
    interleaved-pair broadcast AP (hits the DVE 2x mode).  The dropped
    +su' makes the matmul short by sum_e f3W[n,e]su'[e,i]; that rank-1
    correction C is precomputed on the PE and folded into the final
    copy's per-partition bias.
  p=6,7 -> ACT: true h = relu(tv + su') via activation bias.
Final: one ACT copy [128,512] per octet (software-pipelined one octet
behind the matmuls), then 4 output DMAs of [24, 512].

DMA-instruction issue costs ~600ns regardless of size, so every
multi-chunk tensor is loaded with ONE DMA from a host-prepacked layout
(chunk-major in the free dim), and all small tensors ride in one
"misc" tensor per dtype.
"""

import sys

sys.path.insert(0, "/opt/trn_rl_repo")

import numpy as np

B, L, H, E, N = 2, 512, 768, 256, 12
EC = E // 128  # 2 e-chunks
HC = H // 128  # 6 h-chunks
IB = L // 4  # 128 i's per core
NCORES = 8
OCTS = IB // 8  # 16

# misc fp32 tensor column layout: [sb(2) tb(2) f2b(2) f3b128(1) kxn01(16)
#                                  mask8(8) f3WT(24)]
MISC_W = 2 + 2 + 2 + 1 + OCTS + 8 + 2 * N

_cache = {}


def build_nc():
    import concourse.bass as bass
    import concourse.tile as tile
    from concourse import bacc, mybir
    from contextlib import ExitStack

    fp32 = mybir.dt.float32
    fp16 = mybir.dt.float16
    AF = mybir.ActivationFunctionType
    ALU = mybir.AluOpType

    nc = bacc.Bacc("TRN2")

    # ---- I/O (all multi-chunk tensors prepacked chunk-major on host) ----
    xTm = nc.dram_tensor("xTm", [128, HC * L], fp32, kind="ExternalInput")
    tWTm = nc.dram_tensor("tWTm", [128, HC * E], fp32, kind="ExternalInput")
    xTim = nc.dram_tensor("xTim", [128, HC * IB], fp32, kind="ExternalInput")
    sWTm = nc.dram_tensor("sWTm", [128, HC * E], fp32, kind="ExternalInput")
    WuTm = nc.dram_tensor("WuTm", [128, EC * E], fp16, kind="ExternalInput")
    WvTm = nc.dram_tensor("WvTm", [128, EC * E], fp16, kind="ExternalInput")
    blWTm = nc.dram_tensor("blWTm", [128, EC * E * N], fp16, kind="ExternalInput")
    f3padm = nc.dram_tensor("f3padm", [128, EC * 48], fp16, kind="ExternalInput")
    misc = nc.dram_tensor("misc", [128, MISC_W], fp32, kind="ExternalInput")
    out = nc.dram_tensor("out", [IB, N, L], fp32, kind="ExternalOutput")

    with tile.TileContext(nc) as tc, ExitStack() as ctx:
        consts = ctx.enter_context(tc.tile_pool(name="consts", bufs=1))
        acts = ctx.enter_context(tc.tile_pool(name="acts", bufs=1))

        def load(src, shape, name, dt=fp32, eng=None):
            t = consts.tile(shape, dt, name=name)
            (eng or nc.sync).dma_start(out=t[:], in_=src)
            return t

        # queue order matters: first-needed first per queue
        xT_m = load(xTm[:], [128, HC * L], "xT_m")
        tWT_m = load(tWTm[:], [128, HC * E], "tWT_m")
        xTi_m = load(xTim[:], [128, HC * IB], "xTi_m", eng=nc.gpsimd)
        sWT_m = load(sWTm[:], [128, HC * E], "sWT_m", eng=nc.gpsimd)
        misc_sb = load(misc[:], [128, MISC_W], "misc_sb", eng=nc.gpsimd)
        WuT_m = load(WuTm[:], [128, EC * E], "WuT_m", dt=fp16, eng=nc.scalar)
        WvT_m = load(WvTm[:], [128, EC * E], "WvT_m", dt=fp16, eng=nc.scalar)
        f3pad_m = load(f3padm[:], [128, EC * 48], "f3pad_m", dt=fp16, eng=nc.scalar)
        blWT_m = load(blWTm[:], [128, EC * E * N], "blWT_m", dt=fp16, eng=nc.scalar)

        xT_sb = [xT_m[:, L * c : L * (c + 1)] for c in range(HC)]
        tWT_sb = [tWT_m[:, E * c : E * (c + 1)] for c in range(HC)]
        xTi_sb = [xTi_m[:, IB * c : IB * (c + 1)] for c in range(HC)]
        sWT_sb = [sWT_m[:, E * c : E * (c + 1)] for c in range(HC)]
        WuT_sb = [WuT_m[:, E * c : E * (c + 1)] for c in range(EC)]
        WvT_sb = [WvT_m[:, E * c : E * (c + 1)] for c in range(EC)]
        blWT_sb = [blWT_m[:, E * N * c : E * N * (c + 1)] for c in range(EC)]
        f3pad_sb = [f3pad_m[:, 48 * c : 48 * (c + 1)] for c in range(EC)]
        o_ = 0
        sb_sb = misc_sb[:, o_ : o_ + 2]; o_ += 2
        tb_sb = misc_sb[:, o_ : o_ + 2]; o_ += 2
        f2b_sb = misc_sb[:, o_ : o_ + 2]; o_ += 2
        f3b_sb = misc_sb[:, o_ : o_ + 1]; o_ += 1
        kxn01_sb = misc_sb[:, o_ : o_ + OCTS]; o_ += OCTS
        mask8_sb = misc_sb[:, o_ : o_ + 8]; o_ += 8
        f3WT_sb = [misc_sb[:, o_ + N * c : o_ + N * (c + 1)] for c in range(EC)]

        # ---- persistent activations ----
        tT_sb, sT_sb, sTb_sb, suT_sb, keyE_sb = [], [], [], [], []
        for ec in range(EC):
            tT_sb.append(acts.tile([128, L], fp16, name=f"tT{ec}"))
            sT_sb.append(acts.tile([128, IB], fp32, name=f"sT{ec}"))
            sTb_sb.append(acts.tile([128, IB], fp16, name=f"sTb{ec}"))
            suT_sb.append(acts.tile([128, IB], fp32, name=f"suT{ec}"))
            # key, packed: col 32*d + 12*s + n  (i = 2d+s), pads zero
            keyE_sb.append(acts.tile([128, 32 * 64], fp16, name=f"keyE_{ec}"))
            nc.gpsimd.memset(keyE_sb[ec][:], 0.0)
        tvT2c = acts.tile([128, 2 * L], fp16, name="tvT2c")  # cols 512*ec+j
        tvT2i = acts.tile([128, 2 * L], fp16, name="tvT2i")  # cols 2*j+ec
        negsu2 = acts.tile([128, 2 * IB], fp16, name="negsu2")  # col 2i+ec
        C8sb = acts.tile([128, OCTS], fp32, name="C8sb")
        CT_sb = acts.tile([128, N], fp32, name="CT_sb")
        kxmC = acts.tile([128, 128], fp32, name="kxmC")
        nc.gpsimd.memset(kxmC[:], 0.0)

        # ---- prep ----
        with tc.tile_pool(name="prep_psum", bufs=3, space="PSUM") as pp:
            for ec in range(EC):
                # tT = relu(x @ tW.T + tb)  (fp32 matmul, fp16 out)
                ps_t = pp.tile([128, L], fp32, name="ps_t", tag="ps")
                for hc in range(HC):
                    nc.tensor.matmul(
                        ps_t[:],
                        lhsT=tWT_sb[hc][:, 128 * ec : 128 * (ec + 1)],
                        rhs=xT_sb[hc],
                        start=(hc == 0),
                        stop=(hc == HC - 1),
                    )
                nc.scalar.activation(tT_sb[ec][:], ps_t[:], AF.Relu,
                                     bias=tb_sb[:, ec : ec + 1])

                ps_s = pp.tile([128, L], fp32, name="ps_s", tag="ps")
                for hc in range(HC):
                    nc.tensor.matmul(
                        ps_s[:, :IB],
                        lhsT=sWT_sb[hc][:, 128 * ec : 128 * (ec + 1)],
                        rhs=xTi_sb[hc],
                        start=(hc == 0),
                        stop=(hc == HC - 1),
                    )
                nc.scalar.activation(sT_sb[ec][:], ps_s[:, :IB], AF.Relu,
                                     bias=sb_sb[:, ec : ec + 1])
                nc.vector.tensor_copy(out=sTb_sb[ec][:], in_=sT_sb[ec][:])

            for ec in range(EC):
                # tvT chunk (fp16 matmul) -> both layouts
                ps_tv = pp.tile([128, L], fp32, name="ps_tv", tag="ps")
                for epc in range(EC):
                    nc.tensor.matmul(
                        ps_tv[:],
                        lhsT=WvT_sb[epc][:, 128 * ec : 128 * (ec + 1)],
                        rhs=tT_sb[epc][:],
                        start=(epc == 0),
                        stop=(epc == EC - 1),
                    )
                nc.scalar.copy(tvT2c[:, L * ec : L * (ec + 1)], ps_tv[:])
                tv_i = tvT2i.rearrange("p (f c) -> p f c", c=2)
                nc.vector.tensor_copy(out=tv_i[:, :, ec], in_=ps_tv[:])

                # suT = s @ Wu.T + f2b (fp16 matmul, fp32 out)
                ps_su = pp.tile([128, L], fp32, name="ps_su", tag="ps")
                for epc in range(EC):
                    nc.tensor.matmul(
                        ps_su[:, :IB],
                        lhsT=WuT_sb[epc][:, 128 * ec : 128 * (ec + 1)],
                        rhs=sTb_sb[epc][:],
                        start=(epc == 0),
                        stop=(epc == EC - 1),
                    )
                nc.scalar.activation(suT_sb[ec][:], ps_su[:, :IB], AF.Identity,
                                     bias=f2b_sb[:, ec : ec + 1])
                ns2 = negsu2.rearrange("p (i two) -> p i two", two=2)
                nc.vector.tensor_scalar_mul(ns2[:, :, ec], suT_sb[ec][:], -1.0)

            # correction CT[i, n] = sum_e f3WT[e,n] * suT[e,i]  (fp32)
            ps_ct = pp.tile([128, L], fp32, name="ps_ct", tag="ps")
            for ec in range(EC):
                nc.tensor.matmul(
                    ps_ct[:, :N],
                    lhsT=suT_sb[ec][:],
                    rhs=f3WT_sb[ec],
                    start=(ec == 0),
                    stop=(ec == EC - 1),
                )
            nc.vector.tensor_copy(out=CT_sb[:], in_=ps_ct[:, :N])
            # kxmC[:, 32k+12s+n] = CT[:, n] * (i%8 == 2k+s), p<6 only
            for k in range(4):
                for s in range(2):
                    p = 2 * k + s
                    if p >= 6:
                        continue
                    nc.vector.tensor_tensor(
                        out=kxmC[:, 32 * k + 12 * s : 32 * k + 12 * s + N],
                        in0=CT_sb[:],
                        in1=mask8_sb[:, p : p + 1].broadcast_to([128, N]),
                        op=ALU.mult,
                    )
            ps_c8 = pp.tile([128, L], fp32, name="ps_c8", tag="ps")
            nc.tensor.matmul(ps_c8[:, :OCTS], lhsT=kxmC[:], rhs=kxn01_sb,
                             start=True, stop=True)
            nc.vector.tensor_tensor(
                out=C8sb[:], in0=ps_c8[:, :OCTS],
                in1=f3b_sb.broadcast_to([128, OCTS]), op=ALU.add)

            # key (fp16 matmul): keyE[ec][e, 32d+12s+n] = key[2d+s, 128ec+e, n]
            blWT3 = [blWT_sb[c].rearrange("p (e n) -> p e n", n=N) for c in range(EC)]
            keyv = [keyE_sb[c].rearrange("p (d q) -> p d q", q=32) for c in range(EC)]
            for ec in range(EC):
                for n in range(N):
                    ps_k = pp.tile([128, L], fp32, name="ps_k", tag="ps")
                    for epc in range(EC):
                        nc.tensor.matmul(
                            ps_k[:, :IB],
                            lhsT=blWT3[epc][:, 128 * ec : 128 * (ec + 1), n],
                            rhs=sTb_sb[epc][:],
                            start=(epc == 0),
                            stop=(epc == EC - 1),
                        )
                    # [128, 64, 2] strided dest (cols 32d + 12s + n)
                    src = ps_k[:, :IB].rearrange("p (d s) -> p d s", s=2)
                    dst = keyv[ec][:, :, n : n + 13 : 12]
                    if n % 2 == 0:
                        nc.vector.tensor_copy(out=dst, in_=src)
                    else:
                        nc.scalar.copy(dst, src)

        # ---- main loop over octets (final copy pipelined 1 octet back) ----
        hp = ctx.enter_context(tc.tile_pool(name="hp", bufs=20))
        outp = ctx.enter_context(tc.tile_pool(name="outp", bufs=4))
        mp = ctx.enter_context(tc.tile_pool(name="main_psum", bufs=5, space="PSUM"))

        pending = None  # (psum_tile, octet)

        def flush(pending):
            ps_prev, o_prev = pending
            ob = outp.tile([128, L], fp32, name="ob")
            nc.scalar.activation(ob[:], ps_prev[:], AF.Identity,
                                 bias=C8sb[:, o_prev : o_prev + 1])
            for k in range(4):
                dma_eng = nc.sync if k % 2 == 0 else nc.gpsimd
                dma_eng.dma_start(
                    out=out[8 * o_prev + 2 * k : 8 * o_prev + 2 * k + 2, :, :],
                    in_=ob[32 * k : 32 * k + 24, :])

        for o in range(OCTS):
            ps = mp.tile([128, L], fp32, name="ps")
            # out1: M=32 per (duo, ec); ec0 initializes the full bank
            for ec in range(EC):
                for k in range(4):
                    d = 4 * o + k
                    nc.tensor.matmul(
                        ps[32 * k : 32 * k + 32, :],
                        lhsT=keyE_sb[ec][:, 32 * d : 32 * d + 32],
                        rhs=tT_sb[ec][:],
                        start=(ec == 0),
                        stop=False,
                        tile_position=(0, 32 * k),
                        skip_group_check=True,
                    )
            # h production: p = 0..5 DVE (h'), p = 6,7 ACT (true h)
            hs = {}
            for p in range(8):
                i = 8 * o + p
                if p < 6:
                    h2 = hp.tile([128, 2 * L], fp16, name="h2", tag="h")
                    in1 = negsu2[:, 2 * i : 2 * i + 2].unsqueeze(1)\
                        .broadcast_to([128, L, 2])
                    nc.vector.tensor_tensor(
                        out=h2[:].rearrange("p (f c) -> p f c", c=2),
                        in0=tvT2i[:].rearrange("p (f c) -> p f c", c=2),
                        in1=in1, op=ALU.max)
                    h2v = h2.rearrange("p (f c) -> p f c", c=2)
                    for ec in range(EC):
                        hs[(p, ec)] = h2v[:, :, ec]
                else:
                    for ec in range(EC):
                        ha = hp.tile([128, L], fp16, name="ha", tag="h")
                        nc.scalar.activation(ha[:], tvT2c[:, L * ec : L * (ec + 1)],
                                             AF.Relu, bias=suT_sb[ec][:, i : i + 1])
                        hs[(p, ec)] = ha[:]
            # out2: M=24 zero-block-padded f3 stationaries; emission order
            # rotates col-groups for PE tile concurrency
            for ec in range(EC):
                for p in (0, 2, 4, 6, 1, 3, 5, 7):
                    k, s = divmod(p, 2)
                    nc.tensor.matmul(
                        ps[32 * k : 32 * k + 24, :],
                        lhsT=f3pad_sb[ec][:, 24 * s : 24 * s + 24],
                        rhs=hs[(p, ec)],
                        start=False,
                        stop=(ec == EC - 1),
                        tile_position=(0, 32 * k),
                        skip_group_check=True,
                    )
            if pending is not None:
                flush(pending)
            pending = (ps, o)
        flush(pending)

    nc.compile()
    return nc


def _get_nc():
    if "nc" not in _cache:
        _cache["nc"] = build_nc()
    return _cache["nc"]


def _chunk_major(a, nchunks):
    # [128*nchunks, W] -> [128, nchunks*W] with chunk-major free layout
    W = a.shape[1]
    return np.ascontiguousarray(
        a.reshape(nchunks, 128, W).transpose(1, 0, 2).reshape(128, nchunks * W))


def _make_in_maps(inputs):
    x = np.asarray(inputs["x"], np.float32)
    f32 = lambda a: np.asarray(a, np.float32)

    f2W = f32(inputs["f2W"])
    f3WT = f32(inputs["f3W"]).T  # [E, N]
    f3pad = np.zeros((E, 48), np.float32)
    for s in range(2):
        # slice s covers psum rows 32k..32k+24; i with s=i%2 lands at +12*s
        f3pad[:, 24 * s + 12 * s : 24 * s + 12 * s + N] = f3WT

    misc = np.zeros((128, MISC_W), np.float32)
    o_ = 0
    misc[:, o_ : o_ + 2] = f32(inputs["sb"]).reshape(EC, 128).T; o_ += 2
    misc[:, o_ : o_ + 2] = f32(inputs["tb"]).reshape(EC, 128).T; o_ += 2
    misc[:, o_ : o_ + 2] = f32(inputs["f2b"]).reshape(EC, 128).T; o_ += 2
    for k in range(4):
        for s in range(2):
            misc[32 * k + 12 * s : 32 * k + 12 * s + N, o_] = f32(inputs["f3b"])
    o_ += 1
    for i in range(128):
        if i % 8 < 6:
            misc[i, o_ + i // 8] = 1.0
    o_ += OCTS
    for i in range(128):
        misc[i, o_ + i % 8] = 1.0
    o_ += 8
    misc[:, o_:] = _chunk_major(f3WT, EC)

    shared = {
        "sWTm": _chunk_major(f32(inputs["sW"]).T, HC),
        "tWTm": _chunk_major(f32(inputs["tW"]).T, HC),
        "WuTm": _chunk_major(f2W[:, :E].T, EC).astype(np.float16),
        "WvTm": _chunk_major(f2W[:, E:].T, EC).astype(np.float16),
        "blWTm": _chunk_major(f32(inputs["blW"]).T, EC).astype(np.float16),
        "f3padm": _chunk_major(f3pad, EC).astype(np.float16),
        "misc": misc,
    }

    in_maps = []
    for c in range(NCORES):
        b, r = divmod(c, 4)
        m = dict(shared)
        m["xTm"] = _chunk_major(np.ascontiguousarray(x[b].T), HC)
        m["xTim"] = _chunk_major(
            np.ascontiguousarray(x[b, IB * r : IB * (r + 1), :].T), HC)
        in_maps.append(m)
    return in_maps


def _gather(results):
    full = np.empty((B, L, N, L), np.float32)
    for c in range(NCORES):
        b, r = divmod(c, 4)
        full[b, IB * r : IB * (r + 1)] = results[c]["out"]
    return full


def kernel(x, sW, sb, tW, tb, f2W, f2b, f3W, f3b, blW):
    from concourse.bass_utils import run_bass_kernel_spmd

    in_maps = _make_in_maps(dict(
        x=x, sW=sW, sb=sb, tW=tW, tb=tb, f2W=f2W, f2b=f2b,
        f3W=f3W, f3b=f3b, blW=blW,
    ))
    nc = _get_nc()
    res = run_bass_kernel_spmd(nc, in_maps, core_ids=list(range(NCORES)))
    return _gather(res.results)



# revision 7
# speedup vs baseline: 1.1922x; 1.1922x over previous
"""BiAffine layer kernel for 8 Trainium2 NeuronCores.

Reference computation (per batch b):
  s = relu(x @ sW.T + sb)                  [L, E]
  t = relu(x @ tW.T + tb)                  [L, E]
  key = (s @ blW.T).reshape(L, E, N)
  out1[i, n, l] = sum_e key[i, e, n] * t[l, e]
  su = s @ Wu.T ; tv = t @ Wv.T + f2b      (Wu, Wv = f2W[:, :E], f2W[:, E:])
  h[i, j, :] = relu(su[i] + tv[j])
  out2[i, n, j] = sum_e h[i, j, e] * f3W[n, e] + f3b[n]
  out = out1 + out2                        [L, N, L]

Sharding: 8 cores = 2 batches x 4 blocks of 128 source positions (i).

Octet layout: one PSUM bank [128, 512] holds EIGHT i's: 4 col-groups at
32-aligned offsets, 2 i's packed per group (rows 32k + 12s + n, 8 pad
rows per group).  out1: M=32 matmuls from a zero-padded fp16 key tensor
(also initializes the bank); out2: M=24 matmuls with zero-block-padded
f3W stationaries, 4-way PE column-group concurrency throughout.

h production per (i, ec): ONE fused tensor_scalar
  h = max(tvT[ec] + suT[ec][:, i], 0)
which hits the DVE 4x mode (fp16, SBUF, packed).  Slots are split
DVE/ACT/Pool per octet to balance engines; ACT slots use activation
Relu with a per-partition bias instead.
Final: one ACT copy [128,512] per octet with bias f3b128 (software-
pipelined one octet behind the matmuls), then ONE 4D-AP output DMA.

All inputs are fp16 (except small biases) and loaded with few DMAs
from host-prepacked chunk-major layouts.
"""

import sys

sys.path.insert(0, "/opt/trn_rl_repo")

import numpy as np

B, L, H, E, N = 2, 512, 768, 256, 12
EC = E // 128  # 2 e-chunks
HC = H // 128  # 6 h-chunks
IB = L // 4  # 128 i's per core
NCORES = 8
OCTS = IB // 8  # 16

# misc fp32 tensor column layout: [sb(2) tb(2) f2b(2) f3b128(1)]
MISC_W = 2 + 2 + 2 + 1

_cache = {}


def build_nc():
    import concourse.bass as bass
    import concourse.tile as tile
    from concourse import bacc, mybir
    from contextlib import ExitStack

    fp32 = mybir.dt.float32
    fp16 = mybir.dt.float16
    AF = mybir.ActivationFunctionType
    ALU = mybir.AluOpType

    nc = bacc.Bacc("TRN2")

    # ---- I/O (all multi-chunk tensors prepacked chunk-major on host) ----
    tWTm = nc.dram_tensor("tWTm", [128, HC * E], fp16, kind="ExternalInput")
    xTam = nc.dram_tensor("xTam", [128, 3 * L], fp16, kind="ExternalInput")
    xTbm = nc.dram_tensor("xTbm", [128, 3 * L], fp16, kind="ExternalInput")
    sWTm = nc.dram_tensor("sWTm", [128, HC * E], fp16, kind="ExternalInput")
    WuTm = nc.dram_tensor("WuTm", [128, EC * E], fp16, kind="ExternalInput")
    WvTm = nc.dram_tensor("WvTm", [128, EC * E], fp16, kind="ExternalInput")
    blWTm = nc.dram_tensor("blWTm", [128, EC * E * N], fp16, kind="ExternalInput")
    f3padm = nc.dram_tensor("f3padm", [128, EC * 48], fp16, kind="ExternalInput")
    misc = nc.dram_tensor("misc", [128, MISC_W], fp32, kind="ExternalInput")
    out = nc.dram_tensor("out", [IB, N, L], fp32, kind="ExternalOutput")

    with tile.TileContext(nc) as tc, ExitStack() as ctx:
        consts = ctx.enter_context(tc.tile_pool(name="consts", bufs=1))
        acts = ctx.enter_context(tc.tile_pool(name="acts", bufs=1))

        def load(src, shape, name, dt=fp16, eng=None):
            t = consts.tile(shape, dt, name=name)
            (eng or nc.sync).dma_start(out=t[:], in_=src)
            return t

        # queue order matters: first-needed first per queue
        tWT_m = load(tWTm[:], [128, HC * E], "tWT_m")
        xTa_m = load(xTam[:], [128, 3 * L], "xTa_m")
        xTb_m = load(xTbm[:], [128, 3 * L], "xTb_m")
        sWT_m = load(sWTm[:], [128, HC * E], "sWT_m", eng=nc.scalar)
        misc_sb = load(misc[:], [128, MISC_W], "misc_sb", dt=fp32, eng=nc.scalar)
        WuT_m = load(WuTm[:], [128, EC * E], "WuT_m", eng=nc.scalar)
        WvT_m = load(WvTm[:], [128, EC * E], "WvT_m", eng=nc.scalar)
        f3pad_m = load(f3padm[:], [128, EC * 48], "f3pad_m", eng=nc.scalar)
        blWT_m = load(blWTm[:], [128, EC * E * N], "blWT_m", eng=nc.gpsimd)

        xT_sb = [xTa_m[:, L * c : L * (c + 1)] for c in range(3)] + [
            xTb_m[:, L * c : L * (c + 1)] for c in range(3)
        ]
        # s rhs: cols of this core's i-block within each x chunk (set at
        # runtime by which x slice the host packed -- host packs per-core
        # xTa/xTb already holding the FULL L columns; s uses a col slice
        # chosen by the host via a separate per-core offset baked into the
        # pack).  We bake r into the host pack: s-cols are ALWAYS cols
        # [SOFF, SOFF+IB) of each chunk, with SOFF fixed at pack time.
        tWT_sb = [tWT_m[:, E * c : E * (c + 1)] for c in range(HC)]
        sWT_sb = [sWT_m[:, E * c : E * (c + 1)] for c in range(HC)]
        WuT_sb = [WuT_m[:, E * c : E * (c + 1)] for c in range(EC)]
        WvT_sb = [WvT_m[:, E * c : E * (c + 1)] for c in range(EC)]
        blWT_sb = [blWT_m[:, E * N * c : E * N * (c + 1)] for c in range(EC)]
        f3pad_sb = [f3pad_m[:, 48 * c : 48 * (c + 1)] for c in range(EC)]
        o_ = 0
        sb_sb = misc_sb[:, o_ : o_ + 2]; o_ += 2
        tb_sb = misc_sb[:, o_ : o_ + 2]; o_ += 2
        f2b_sb = misc_sb[:, o_ : o_ + 2]; o_ += 2
        f3b_sb = misc_sb[:, o_ : o_ + 1]; o_ += 1

        # ---- persistent activations ----
        tT_sb, sTb_sb, suT_sb, keyE_sb = [], [], [], []
        for ec in range(EC):
            tT_sb.append(acts.tile([128, L], fp16, name=f"tT{ec}"))
            sTb_sb.append(acts.tile([128, IB], fp16, name=f"sTb{ec}"))
            suT_sb.append(acts.tile([128, IB], fp32, name=f"suT{ec}"))
            # key, packed: col 32*d + 12*s + n  (i = 2d+s), pads zero
            keyE_sb.append(acts.tile([128, 32 * 64], fp16, name=f"keyE_{ec}"))
            nc.gpsimd.memset(keyE_sb[ec][:], 0.0)
        tvTc = acts.tile([128, 2 * L], fp16, name="tvTc")  # cols 512*ec+j

        # ---- prep (pools coexist with main loop for overlap) ----
        pp = ctx.enter_context(tc.tile_pool(name="prep_psum", bufs=3, space="PSUM"))
        for ec in range(EC):
            # tT = relu(x @ tW.T + tb)  (fp16 matmul)
            ps_t = pp.tile([128, L], fp32, name="ps_t", tag="ps")
            for hc in range(HC):
                nc.tensor.matmul(
                    ps_t[:],
                    lhsT=tWT_sb[hc][:, 128 * ec : 128 * (ec + 1)],
                    rhs=xT_sb[hc],
                    start=(hc == 0),
                    stop=(hc == HC - 1),
                )
            nc.scalar.activation(tT_sb[ec][:], ps_t[:], AF.Relu,
                                 bias=tb_sb[:, ec : ec + 1])

            ps_s = pp.tile([128, L], fp32, name="ps_s", tag="ps")
            for hc in range(HC):
                nc.tensor.matmul(
                    ps_s[:, :IB],
                    lhsT=sWT_sb[hc][:, 128 * ec : 128 * (ec + 1)],
                    rhs=xT_sb[hc][:, 0:IB],  # host packs s-cols at offset 0
                    start=(hc == 0),
                    stop=(hc == HC - 1),
                )
            nc.scalar.activation(sTb_sb[ec][:], ps_s[:, :IB], AF.Relu,
                                 bias=sb_sb[:, ec : ec + 1])

        for ec in range(EC):
            # tvT chunk (fp16 matmul), + f2b folded in here
            ps_tv = pp.tile([128, L], fp32, name="ps_tv", tag="ps")
            for epc in range(EC):
                nc.tensor.matmul(
                    ps_tv[:],
                    lhsT=WvT_sb[epc][:, 128 * ec : 128 * (ec + 1)],
                    rhs=tT_sb[epc][:],
                    start=(epc == 0),
                    stop=(epc == EC - 1),
                )
            nc.scalar.activation(tvTc[:, L * ec : L * (ec + 1)], ps_tv[:],
                                 AF.Identity, bias=f2b_sb[:, ec : ec + 1])

            # suT = s @ Wu.T (fp16 matmul, fp32 out)
            ps_su = pp.tile([128, L], fp32, name="ps_su", tag="ps")
            for epc in range(EC):
                nc.tensor.matmul(
                    ps_su[:, :IB],
                    lhsT=WuT_sb[epc][:, 128 * ec : 128 * (ec + 1)],
                    rhs=sTb_sb[epc][:],
                    start=(epc == 0),
                    stop=(epc == EC - 1),
                )
            nc.vector.tensor_copy(out=suT_sb[ec][:], in_=ps_su[:, :IB])

        # key (fp16 matmul): keyE[ec][e, 32d+12s+n] = key[2d+s, 128ec+e, n]
        # 4 n's per PSUM bank, one merged strided copy per (ec, quad)
        blWT3 = [blWT_sb[c].rearrange("p (e n) -> p e n", n=N) for c in range(EC)]
        copy_engs = [nc.vector, nc.scalar, nc.vector, nc.scalar, nc.vector, nc.scalar]
        qi = 0
        for ec in range(EC):
            for q in range(3):
                ps_k = pp.tile([128, L], fp32, name="ps_k", tag="ps")
                for nq in range(4):
                    n = 4 * q + nq
                    for epc in range(EC):
                        nc.tensor.matmul(
                            ps_k[:, 128 * nq : 128 * nq + IB],
                            lhsT=blWT3[epc][:, 128 * ec : 128 * (ec + 1), n],
                            rhs=sTb_sb[epc][:],
                            start=(epc == 0),
                            stop=(epc == EC - 1),
                        )
                # src col 128*nq + 2d + s -> dst col 32d + 12s + 4q + nq
                src = ps_k[:].rearrange("p (nq d s) -> p d s nq", nq=4, s=2)
                dstv = keyE_sb[ec][:].rearrange("p (d c) -> p d c", c=32)
                dst = dstv[:, :, 4 * q : 4 * q + 24].rearrange(
                    "p d (s n) -> p d s n", s=2)[:, :, :, 0:4]
                if qi % 2 == 0:
                    nc.vector.tensor_copy(out=dst, in_=src)
                else:
                    nc.scalar.copy(dst, src)
                qi += 1

        # ---- main loop over octets (final copy pipelined 1 octet back,
        # output DMAs batched over quads of 4 octets) ----
        hp = ctx.enter_context(tc.tile_pool(name="hp", bufs=44))
        outp = ctx.enter_context(tc.tile_pool(name="outp", bufs=2))
        mp = ctx.enter_context(tc.tile_pool(name="main_psum", bufs=4, space="PSUM"))

        # engine per (octet position p, ec): DVE except three
        HENG = {(6, 0): "pool", (6, 1): "pool", (7, 0): "pool", (7, 1): "act"}

        outv = out.rearrange("(oo r) n j -> oo r n j", r=8)
        pending = None  # (psum_tile, octet)
        ob4 = [None]

        def flush(pending):
            ps_prev, o_prev = pending
            oq = o_prev % 4
            if oq == 0:
                ob4[0] = outp.tile([128, 4 * L], fp32, name="ob4")
            ob = ob4[0]
            nc.scalar.activation(ob[:, L * oq : L * (oq + 1)], ps_prev[:],
                                 AF.Identity, bias=f3b_sb)
            if oq == 3:
                base = o_prev - 3
                for k in range(4):
                    for s in range(2):
                        sA = ob[32 * k + 12 * s : 32 * k + 12 * s + 12, :]\
                            .rearrange("n (oo j) -> n oo j", oo=4)
                        dA = outv[base : base + 4, 2 * k + s, :, :]\
                            .rearrange("oo n j -> n oo j")
                        eng = nc.sync if k < 2 else nc.scalar
                        eng.dma_start(out=dA, in_=sA)

        for o in range(OCTS):
            ps = mp.tile([128, L], fp32, name="ps")
            # out1: M=32 per (duo, ec); ec0 initializes the full bank
            for ec in range(EC):
                for k in range(4):
                    d = 4 * o + k
                    nc.tensor.matmul(
                        ps[32 * k : 32 * k + 32, :],
                        lhsT=keyE_sb[ec][:, 32 * d : 32 * d + 32],
                        rhs=tT_sb[ec][:],
                        start=(ec == 0),
                        stop=False,
                        tile_position=(0, 32 * k),
                        skip_group_check=True,
                    )
            # h production: fused relu(tv + su_i) per (p, ec)
            hs = {}
            for p in range(8):
                i = 8 * o + p
                for ec in range(EC):
                    ht = hp.tile([128, L], fp16, name="ht", tag="h")
                    eng = HENG.get(p, "dve")
                    if eng == "act":
                        nc.scalar.activation(ht[:], tvTc[:, L * ec : L * (ec + 1)],
                                             AF.Relu, bias=suT_sb[ec][:, i : i + 1])
                    else:
                        e = nc.gpsimd if eng == "pool" else nc.vector
                        e.tensor_scalar(
                            out=ht[:],
                            in0=tvTc[:, L * ec : L * (ec + 1)],
                            scalar1=suT_sb[ec][:, i : i + 1],
                            scalar2=0.0,
                            op0=ALU.add,
                            op1=ALU.max,
                        )
                    hs[(p, ec)] = ht[:]
            # out2: M=24 zero-block-padded f3 stationaries; emission order
            # rotates col-groups for PE tile concurrency
            for ec in range(EC):
                for p in (0, 2, 4, 6, 1, 3, 5, 7):
                    k, s = divmod(p, 2)
                    nc.tensor.matmul(
                        ps[32 * k : 32 * k + 24, :],
                        lhsT=f3pad_sb[ec][:, 24 * s : 24 * s + 24],
                        rhs=hs[(p, ec)],
                        start=False,
                        stop=(ec == EC - 1),
                        tile_position=(0, 32 * k),
                        skip_group_check=True,
                    )
            if pending is not None:
                flush(pending)
            pending = (ps, o)
        flush(pending)

    nc.compile()
    return nc


def _get_nc():
    if "nc" not in _cache:
        _cache["nc"] = build_nc()
    return _cache["nc"]


def _chunk_major(a, nchunks):
    # [128*nchunks, W] -> [128, nchunks*W] with chunk-major free layout
    W = a.shape[1]
    return np.ascontiguousarray(
        a.reshape(nchunks, 128, W).transpose(1, 0, 2).reshape(128, nchunks * W))


def _make_in_maps(inputs):
    x = np.asarray(inputs["x"], np.float32)
    f32 = lambda a: np.asarray(a, np.float32)
    f16 = np.float16

    f2W = f32(inputs["f2W"])
    f3WT = f32(inputs["f3W"]).T  # [E, N]
    f3pad = np.zeros((E, 48), np.float32)
    for s in range(2):
        # slice s covers psum rows 32k..32k+24; i with s=i%2 lands at +12*s
        f3pad[:, 24 * s + 12 * s : 24 * s + 12 * s + N] = f3WT

    misc = np.zeros((128, MISC_W), np.float32)
    o_ = 0
    misc[:, o_ : o_ + 2] = f32(inputs["sb"]).reshape(EC, 128).T; o_ += 2
    misc[:, o_ : o_ + 2] = f32(inputs["tb"]).reshape(EC, 128).T; o_ += 2
    misc[:, o_ : o_ + 2] = f32(inputs["f2b"]).reshape(EC, 128).T; o_ += 2
    for k in range(4):
        for s in range(2):
            misc[32 * k + 12 * s : 32 * k + 12 * s + N, o_] = f32(inputs["f3b"])
    o_ += 1

    shared = {
        "sWTm": _chunk_major(f32(inputs["sW"]).T, HC).astype(f16),
        "tWTm": _chunk_major(f32(inputs["tW"]).T, HC).astype(f16),
        "WuTm": _chunk_major(f2W[:, :E].T, EC).astype(f16),
        "WvTm": _chunk_major(f2W[:, E:].T, EC).astype(f16),
        "blWTm": _chunk_major(f32(inputs["blW"]).T, EC).astype(f16),
        "f3padm": _chunk_major(f3pad, EC).astype(f16),
        "misc": misc,
    }

    in_maps = []
    for c in range(NCORES):
        b, r = divmod(c, 4)
        m = dict(shared)
        # x chunks, with this core's 128 i-columns rotated to the front of
        # each chunk so the s matmul reads cols [0, IB) of every chunk
        xT = np.ascontiguousarray(x[b].T)  # [H, L]
        xTr = np.roll(xT, -IB * r, axis=1)
        xm = _chunk_major(xTr, HC).astype(f16)  # [128, HC*L]
        m["xTam"] = np.ascontiguousarray(xm[:, : 3 * L])
        m["xTbm"] = np.ascontiguousarray(xm[:, 3 * L :])
        in_maps.append(m)
    return in_maps


def _gather(results):
    full = np.empty((B, L, N, L), np.float32)
    for c in range(NCORES):
        b, r = divmod(c, 4)
        # per-core x columns were rolled by -IB*r, so the last axis (l)
        # of this core's output is rolled too; undo it here
        full[b, IB * r : IB * (r + 1)] = np.roll(results[c]["out"], IB * r, axis=-1)
    return full


def kernel(x, sW, sb, tW, tb, f2W, f2b, f3W, f3b, blW):
    from concourse.bass_utils import run_bass_kernel_spmd

    in_maps = _make_in_maps(dict(
        x=x, sW=sW, sb=sb, tW=tW, tb=tb, f2W=f2W, f2b=f2b,
        f3W=f3W, f3b=f3b, blW=blW,
    ))
    nc = _get_nc()
    res = run_bass_kernel_spmd(nc, in_maps, core_ids=list(range(NCORES)))
    return _gather(res.results)


# revision 14
# speedup vs baseline: 1.2434x; 1.0429x over previous
"""BiAffine layer kernel for 8 Trainium2 NeuronCores.

Reference computation (per batch b):
  s = relu(x @ sW.T + sb)                  [L, E]
  t = relu(x @ tW.T + tb)                  [L, E]
  key = (s @ blW.T).reshape(L, E, N)
  out1[i, n, l] = sum_e key[i, e, n] * t[l, e]
  su = s @ Wu.T ; tv = t @ Wv.T + f2b      (Wu, Wv = f2W[:, :E], f2W[:, E:])
  h[i, j, :] = relu(su[i] + tv[j])
  out2[i, n, j] = sum_e h[i, j, e] * f3W[n, e] + f3b[n]
  out = out1 + out2                        [L, N, L]

Sharding: 8 cores = 2 batches x 4 blocks of 128 source positions (i).

Octet layout: one PSUM bank [128, 512] holds EIGHT i's: 4 col-groups at
32-aligned offsets, 2 i's packed per group (rows 32k + 12s + n, 8 pad
rows per group).  out1: M=32 matmuls from a zero-padded fp16 key tensor
(also initializes the bank); out2: M=24 matmuls with zero-block-padded
f3W stationaries, 4-way PE column-group concurrency throughout.

h production per (i, ec): ONE fused tensor_scalar
  h = max(tvT[ec] + suT[ec][:, i], 0)
which hits the DVE 4x mode (fp16, SBUF, packed).  Slots are split
DVE/ACT/Pool per octet to balance engines; ACT slots use activation
Relu with a per-partition bias instead.
Final: one ACT copy [128,512] per octet with bias f3b128 (software-
pipelined one octet behind the matmuls), then ONE 4D-AP output DMA.

All inputs are fp16 (except small biases) and loaded with few DMAs
from host-prepacked chunk-major layouts.
"""

import sys

sys.path.insert(0, "/opt/trn_rl_repo")

import numpy as np

B, L, H, E, N = 2, 512, 768, 256, 12
EC = E // 128  # 2 e-chunks
HC = H // 128  # 6 h-chunks
IB = L // 4  # 128 i's per core
NCORES = 8
OCTS = IB // 8  # 16

# misc fp32 tensor column layout: [sb(2) tb(2) f2b(2) f3b128(1)]
MISC_W = 2 + 2 + 2 + 1

_cache = {}


def build_nc():
    import concourse.bass as bass
    import concourse.tile as tile
    from concourse import bacc, mybir
    from contextlib import ExitStack

    fp32 = mybir.dt.float32
    fp16 = mybir.dt.float16
    AF = mybir.ActivationFunctionType
    ALU = mybir.AluOpType

    nc = bacc.Bacc("TRN2")

    # ---- I/O (all multi-chunk tensors prepacked chunk-major on host) ----
    tWTm = nc.dram_tensor("tWTm", [128, HC * E], fp16, kind="ExternalInput")
    xTam = nc.dram_tensor("xTam", [128, 3 * L], fp16, kind="ExternalInput")
    xTbm = nc.dram_tensor("xTbm", [128, 3 * L], fp16, kind="ExternalInput")
    sWTm = nc.dram_tensor("sWTm", [128, HC * E], fp16, kind="ExternalInput")
    WuTm = nc.dram_tensor("WuTm", [128, EC * E], fp16, kind="ExternalInput")
    WvTm = nc.dram_tensor("WvTm", [128, EC * E], fp16, kind="ExternalInput")
    blWTm = nc.dram_tensor("blWTm", [128, EC * E * N], fp16, kind="ExternalInput")
    f3padm = nc.dram_tensor("f3padm", [128, EC * 48], fp16, kind="ExternalInput")
    misc = nc.dram_tensor("misc", [128, MISC_W], fp32, kind="ExternalInput")
    out = nc.dram_tensor("out", [IB, N, L], fp16, kind="ExternalOutput")

    with tile.TileContext(nc) as tc, ExitStack() as ctx:
        consts = ctx.enter_context(tc.tile_pool(name="consts", bufs=1))
        acts = ctx.enter_context(tc.tile_pool(name="acts", bufs=1))

        def load(src, shape, name, dt=fp16, eng=None):
            t = consts.tile(shape, dt, name=name)
            (eng or nc.sync).dma_start(out=t[:], in_=src)
            return t

        # queue order matters: first-needed first per queue
        xTa_m = load(xTam[:], [128, 3 * L], "xTa_m")
        xTb_m = load(xTbm[:], [128, 3 * L], "xTb_m")
        tWT_m = load(tWTm[:], [128, HC * E], "tWT_m", eng=nc.scalar)
        sWT_m = load(sWTm[:], [128, HC * E], "sWT_m", eng=nc.scalar)
        misc_sb = load(misc[:], [128, MISC_W], "misc_sb", dt=fp32, eng=nc.scalar)
        WuT_m = load(WuTm[:], [128, EC * E], "WuT_m", eng=nc.scalar)
        WvT_m = load(WvTm[:], [128, EC * E], "WvT_m", eng=nc.scalar)
        f3pad_m = load(f3padm[:], [128, EC * 48], "f3pad_m", eng=nc.scalar)
        blWT_m = load(blWTm[:], [128, EC * E * N], "blWT_m", eng=nc.gpsimd)

        xT_sb = [xTa_m[:, L * c : L * (c + 1)] for c in range(3)] + [
            xTb_m[:, L * c : L * (c + 1)] for c in range(3)
        ]
        # s rhs: cols of this core's i-block within each x chunk (set at
        # runtime by which x slice the host packed -- host packs per-core
        # xTa/xTb already holding the FULL L columns; s uses a col slice
        # chosen by the host via a separate per-core offset baked into the
        # pack).  We bake r into the host pack: s-cols are ALWAYS cols
        # [SOFF, SOFF+IB) of each chunk, with SOFF fixed at pack time.
        tWT_sb = [tWT_m[:, E * c : E * (c + 1)] for c in range(HC)]
        sWT_sb = [sWT_m[:, E * c : E * (c + 1)] for c in range(HC)]
        WuT_sb = [WuT_m[:, E * c : E * (c + 1)] for c in range(EC)]
        WvT_sb = [WvT_m[:, E * c : E * (c + 1)] for c in range(EC)]
        blWT_sb = [blWT_m[:, E * N * c : E * N * (c + 1)] for c in range(EC)]
        f3pad_sb = [f3pad_m[:, 48 * c : 48 * (c + 1)] for c in range(EC)]
        o_ = 0
        sb_sb = misc_sb[:, o_ : o_ + 2]; o_ += 2
        tb_sb = misc_sb[:, o_ : o_ + 2]; o_ += 2
        f2b_sb = misc_sb[:, o_ : o_ + 2]; o_ += 2
        f3b_sb = misc_sb[:, o_ : o_ + 1]; o_ += 1

        # ---- persistent activations (memsets AFTER the gpsimd dma issue) ----
        tT_sb, sTb_sb, suT_sb, keyE_sb = [], [], [], []
        for ec in range(EC):
            tT_sb.append(acts.tile([128, L], fp16, name=f"tT{ec}"))
            sTb_sb.append(acts.tile([128, IB], fp16, name=f"sTb{ec}"))
            suT_sb.append(acts.tile([128, IB], fp32, name=f"suT{ec}"))
            # key, packed: col 32*d + 12*s + n  (i = 2d+s), pads zero
            keyE_sb.append(acts.tile([128, 32 * 64], fp16, name=f"keyE_{ec}"))
        for ec in range(EC):
            nc.gpsimd.memset(keyE_sb[ec][:], 0.0)
        tvTc = acts.tile([128, 2 * L], fp16, name="tvTc")  # cols 512*ec+j

        # ---- prep (pools coexist with main loop for overlap) ----
        pp = ctx.enter_context(tc.tile_pool(name="prep_psum", bufs=3, space="PSUM"))
        for ec in range(EC):
            # tT = relu(x @ tW.T + tb)  (fp16 matmul)
            ps_t = pp.tile([128, L], fp32, name="ps_t", tag="ps")
            for hc in range(HC):
                nc.tensor.matmul(
                    ps_t[:],
                    lhsT=tWT_sb[hc][:, 128 * ec : 128 * (ec + 1)],
                    rhs=xT_sb[hc],
                    start=(hc == 0),
                    stop=(hc == HC - 1),
                )
            nc.scalar.activation(tT_sb[ec][:], ps_t[:], AF.Relu,
                                 bias=tb_sb[:, ec : ec + 1])

            ps_s = pp.tile([128, L], fp32, name="ps_s", tag="ps")
            for hc in range(HC):
                nc.tensor.matmul(
                    ps_s[:, :IB],
                    lhsT=sWT_sb[hc][:, 128 * ec : 128 * (ec + 1)],
                    rhs=xT_sb[hc][:, 0:IB],  # host packs s-cols at offset 0
                    start=(hc == 0),
                    stop=(hc == HC - 1),
                )
            nc.scalar.activation(sTb_sb[ec][:], ps_s[:, :IB], AF.Relu,
                                 bias=sb_sb[:, ec : ec + 1])

        for ec in range(EC):
            # tvT chunk (fp16 matmul), + f2b folded in here
            ps_tv = pp.tile([128, L], fp32, name="ps_tv", tag="ps")
            for epc in range(EC):
                nc.tensor.matmul(
                    ps_tv[:],
                    lhsT=WvT_sb[epc][:, 128 * ec : 128 * (ec + 1)],
                    rhs=tT_sb[epc][:],
                    start=(epc == 0),
                    stop=(epc == EC - 1),
                )
            nc.scalar.activation(tvTc[:, L * ec : L * (ec + 1)], ps_tv[:],
                                 AF.Identity, bias=f2b_sb[:, ec : ec + 1])

            # suT = s @ Wu.T (fp16 matmul, fp32 out)
            ps_su = pp.tile([128, L], fp32, name="ps_su", tag="ps")
            for epc in range(EC):
                nc.tensor.matmul(
                    ps_su[:, :IB],
                    lhsT=WuT_sb[epc][:, 128 * ec : 128 * (ec + 1)],
                    rhs=sTb_sb[epc][:],
                    start=(epc == 0),
                    stop=(epc == EC - 1),
                )
            nc.vector.tensor_copy(out=suT_sb[ec][:], in_=ps_su[:, :IB])

        # key (fp16 matmul): keyE[ec][e, 32d+12s+n] = key[2d+s, 128ec+e, n]
        # 4 n's per PSUM bank, one merged strided copy per (ec, quad)
        blWT3 = [blWT_sb[c].rearrange("p (e n) -> p e n", n=N) for c in range(EC)]
        copy_engs = [nc.vector, nc.scalar, nc.vector, nc.scalar, nc.vector, nc.scalar]
        qi = 0
        for ec in range(EC):
            for q in range(3):
                ps_k = pp.tile([128, L], fp32, name="ps_k", tag="ps")
                for nq in range(4):
                    n = 4 * q + nq
                    for epc in range(EC):
                        nc.tensor.matmul(
                            ps_k[:, 128 * nq : 128 * nq + IB],
                            lhsT=blWT3[epc][:, 128 * ec : 128 * (ec + 1), n],
                            rhs=sTb_sb[epc][:],
                            start=(epc == 0),
                            stop=(epc == EC - 1),
                        )
                # src col 128*nq + 2d + s -> dst col 32d + 12s + 4q + nq
                src = ps_k[:].rearrange("p (nq d s) -> p d s nq", nq=4, s=2)
                dstv = keyE_sb[ec][:].rearrange("p (d c) -> p d c", c=32)
                dst = dstv[:, :, 4 * q : 4 * q + 24].rearrange(
                    "p d (s n) -> p d s n", s=2)[:, :, :, 0:4]
                if qi % 2 == 0:
                    nc.vector.tensor_copy(out=dst, in_=src)
                else:
                    nc.scalar.copy(dst, src)
                qi += 1

        # ---- main loop over octets (final copy pipelined 1 octet back,
        # output DMAs batched over quads of 4 octets) ----
        hp = ctx.enter_context(tc.tile_pool(name="hp", bufs=44))
        outp = ctx.enter_context(tc.tile_pool(name="outp", bufs=2))
        mp = ctx.enter_context(tc.tile_pool(name="main_psum", bufs=4, space="PSUM"))

        # engine per (octet position p, ec): DVE except four on ACT
        HENG = {(6, 0): "act", (6, 1): "act", (7, 0): "act", (7, 1): "act"}

        outv = out.rearrange("(oo r) n j -> oo r n j", r=8)
        pending = None  # (psum_tile, octet)
        ob4 = [None]

        def flush(pending):
            ps_prev, o_prev = pending
            oq = o_prev % 4
            if oq == 0:
                ob4[0] = outp.tile([128, 4 * L], fp16, name="ob4")
            ob = ob4[0]
            nc.scalar.activation(ob[:, L * oq : L * (oq + 1)], ps_prev[:],
                                 AF.Identity, bias=f3b_sb)
            if oq == 3:
                base = o_prev - 3
                last = base == OCTS - 4
                engs = ([nc.sync, nc.sync, nc.gpsimd, nc.gpsimd] if not last
                        else [nc.sync, nc.scalar, nc.sync, nc.gpsimd])
                for k in range(4):
                    for s in range(2):
                        sA = ob[32 * k + 12 * s : 32 * k + 12 * s + 12, :]\
                            .rearrange("n (oo j) -> n oo j", oo=4)
                        dA = outv[base : base + 4, 2 * k + s, :, :]\
                            .rearrange("oo n j -> n oo j")
                        engs[k].dma_start(out=dA, in_=sA)

        for o in range(OCTS):
            ps = mp.tile([128, L], fp32, name="ps")
            # out1: M=32 per (duo, ec); ec0 initializes the full bank
            for ec in range(EC):
                for k in range(4):
                    d = 4 * o + k
                    nc.tensor.matmul(
                        ps[32 * k : 32 * k + 32, :],
                        lhsT=keyE_sb[ec][:, 32 * d : 32 * d + 32],
                        rhs=tT_sb[ec][:],
                        start=(ec == 0),
                        stop=False,
                        tile_position=(0, 32 * k),
                        skip_group_check=True,
                    )
            # h production: fused relu(tv + su_i) per (p, ec)
            hs = {}
            for p in range(8):
                i = 8 * o + p
                for ec in range(EC):
                    ht = hp.tile([128, L], fp16, name="ht", tag="h")
                    eng = HENG.get((p, ec), "dve")
                    if eng == "act":
                        nc.scalar.activation(ht[:], tvTc[:, L * ec : L * (ec + 1)],
                                             AF.Relu, bias=suT_sb[ec][:, i : i + 1])
                    else:
                        e = nc.gpsimd if eng == "pool" else nc.vector
                        e.tensor_scalar(
                            out=ht[:],
                            in0=tvTc[:, L * ec : L * (ec + 1)],
                            scalar1=suT_sb[ec][:, i : i + 1],
                            scalar2=0.0,
                            op0=ALU.add,
                            op1=ALU.max,
                        )
                    hs[(p, ec)] = ht[:]
            # out2: M=24 zero-block-padded f3 stationaries; emission order
            # rotates col-groups for PE tile concurrency
            for ec in range(EC):
                for p in (0, 2, 4, 6, 1, 3, 5, 7):
                    k, s = divmod(p, 2)
                    nc.tensor.matmul(
                        ps[32 * k : 32 * k + 24, :],
                        lhsT=f3pad_sb[ec][:, 24 * s : 24 * s + 24],
                        rhs=hs[(p, ec)],
                        start=False,
                        stop=(ec == EC - 1),
                        tile_position=(0, 32 * k),
                        skip_group_check=True,
                    )
            if pending is not None:
                flush(pending)
            pending = (ps, o)
        flush(pending)

    nc.compile()
    return nc


def _get_nc():
    if "nc" not in _cache:
        _cache["nc"] = build_nc()
    return _cache["nc"]


def _chunk_major(a, nchunks):
    # [128*nchunks, W] -> [128, nchunks*W] with chunk-major free layout
    W = a.shape[1]
    return np.ascontiguousarray(
        a.reshape(nchunks, 128, W).transpose(1, 0, 2).reshape(128, nchunks * W))


def _make_in_maps(inputs):
    x = np.asarray(inputs["x"], np.float32)
    f32 = lambda a: np.asarray(a, np.float32)
    f16 = np.float16

    f2W = f32(inputs["f2W"])
    f3WT = f32(inputs["f3W"]).T  # [E, N]
    f3pad = np.zeros((E, 48), np.float32)
    for s in range(2):
        # slice s covers psum rows 32k..32k+24; i with s=i%2 lands at +12*s
        f3pad[:, 24 * s + 12 * s : 24 * s + 12 * s + N] = f3WT

    misc = np.zeros((128, MISC_W), np.float32)
    o_ = 0
    misc[:, o_ : o_ + 2] = f32(inputs["sb"]).reshape(EC, 128).T; o_ += 2
    misc[:, o_ : o_ + 2] = f32(inputs["tb"]).reshape(EC, 128).T; o_ += 2
    misc[:, o_ : o_ + 2] = f32(inputs["f2b"]).reshape(EC, 128).T; o_ += 2
    for k in range(4):
        for s in range(2):
            misc[32 * k + 12 * s : 32 * k + 12 * s + N, o_] = f32(inputs["f3b"])
    o_ += 1

    shared = {
        "sWTm": _chunk_major(f32(inputs["sW"]).T, HC).astype(f16),
        "tWTm": _chunk_major(f32(inputs["tW"]).T, HC).astype(f16),
        "WuTm": _chunk_major(f2W[:, :E].T, EC).astype(f16),
        "WvTm": _chunk_major(f2W[:, E:].T, EC).astype(f16),
        "blWTm": _chunk_major(f32(inputs["blW"]).T, EC).astype(f16),
        "f3padm": _chunk_major(f3pad, EC).astype(f16),
        "misc": misc,
    }

    in_maps = []
    for c in range(NCORES):
        b, r = divmod(c, 4)
        m = dict(shared)
        # x chunks, with this core's 128 i-columns rotated to the front of
        # each chunk so the s matmul reads cols [0, IB) of every chunk
        xT = np.ascontiguousarray(x[b].T)  # [H, L]
        xTr = np.roll(xT, -IB * r, axis=1)
        xm = _chunk_major(xTr, HC).astype(f16)  # [128, HC*L]
        m["xTam"] = np.ascontiguousarray(xm[:, : 3 * L])
        m["xTbm"] = np.ascontiguousarray(xm[:, 3 * L :])
        in_maps.append(m)
    return in_maps


def _gather(results):
    full = np.empty((B, L, N, L), np.float32)
    for c in range(NCORES):
        b, r = divmod(c, 4)
        # per-core x columns were rolled by -IB*r, so the last axis (l)
        # of this core's output is rolled too; undo it here
        full[b, IB * r : IB * (r + 1)] = np.roll(
            results[c]["out"].astype(np.float32), IB * r, axis=-1)
    return full


def kernel(x, sW, sb, tW, tb, f2W, f2b, f3W, f3b, blW):
    from concourse.bass_utils import run_bass_kernel_spmd

    in_maps = _make_in_maps(dict(
        x=x, sW=sW, sb=sb, tW=tW, tb=tb, f2W=f2W, f2b=f2b,
        f3W=f3W, f3b=f3b, blW=blW,
    ))
    nc = _get_nc()
    res = run_bass_kernel_spmd(nc, in_maps, core_ids=list(range(NCORES)))
    return _gather(res.results)
